# revision 12
# baseline (speedup 1.0000x reference)
"""APPNP node classifier on 8 TRN2 NeuronCores — scan-based aggregation.

Strategy:
  - Nodes sharded across 8 cores (6250/core); MLP weights replicated; all
    propagation state kept feature-major [64, nodes].
  - Per-core SBUF table [128, 25088] f32: partition p<64 holds feature p of
    nodes 0..24999 ("lo"), p>=64 feature p-64 of nodes 25000..49999 ("hi");
    table values are dinv[n]*h[n] so edge weights need no per-edge multiply.
  - Per step: gpsimd.ap_gather pulls each edge's source column from the
    table (lo edges on partitions 0-63, hi on 64-127, independent static
    index streams), DVE tensor_tensor_scan builds prefix sums, a second
    small ap_gather reads the prefix at static segment boundaries, and a
    strided subtract yields per-dst-column sums. No DMA descriptors per
    edge, no scatter matmuls.
  - Self-loops via the closed form 0.9*(dinv*(agg + dinv*h)) + 0.1*z.
  - h exchanged between cores as bf16 via AllGather of [64, 6250] blocks;
    table reloaded with casting gpsimd DMAs.
"""

import os
import sys

for _p in ("/opt/trn_rl_repo", "/root/.axon_site/_ro/trn_rl_repo", "/root/.axon_site"):
    if os.path.isdir(_p) and _p not in sys.path:
        sys.path.insert(0, _p)

import numpy as np

NCORES = 8
P = 128
N = 50000
IN_C, HID_C, OUT_C = 512, 256, 64
K = 10
ALPHA = 0.1
NPC = N // NCORES          # 6250
HALF = 25000               # lo/hi node split
NELEM = 25088              # table slots per half (>= HALF, with zero pad)
IDX_ZERO = 25080           # guaranteed-zero table slot (>= HALF)
BLK = 1024                 # edges per stream per block (incl. slot-0 dummy)
NC_CAP = 448               # max dst columns per block
MLP_BLK = 512


def _wrap16(stream, nidx, pad):
    """Wrap a 1-D index stream into ap_gather's [16, nidx//16] layout."""
    s = np.full(nidx, pad, np.int16)
    s[: len(stream)] = stream
    return s.reshape(nidx // 16, 16).T  # [16, nidx//16]


def _prep(edge_index):
    """Host precompute: per-core block cuts, index streams, coefficients."""
    src = np.asarray(edge_index[0], dtype=np.int64)
    dst = np.asarray(edge_index[1], dtype=np.int64)
    E = len(src)

    deg = np.bincount(dst, minlength=N).astype(np.float64) + 1.0
    dinv = (1.0 / np.sqrt(deg)).astype(np.float32)

    # per-core, per-stream edge lists sorted by dst
    per_core = []
    for c in range(NCORES):
        m = (dst >= c * NPC) & (dst < (c + 1) * NPC)
        s_c, d_c = src[m], dst[m] - c * NPC
        order = np.argsort(d_c, kind="stable")
        s_c, d_c = s_c[order], d_c[order]
        lo_m = s_c < HALF
        per_core.append(
            dict(
                lo_src=s_c[lo_m], lo_dst=d_c[lo_m],
                hi_src=s_c[~lo_m] - HALF, hi_dst=d_c[~lo_m],
            )
        )

    # common block cuts over dst columns: both streams of every core must
    # fit in BLK-1 slots, and at most NC_CAP columns per block
    cnt = np.zeros((NCORES, 2, NPC), np.int64)
    for c in range(NCORES):
        cnt[c, 0] = np.bincount(per_core[c]["lo_dst"], minlength=NPC)
        cnt[c, 1] = np.bincount(per_core[c]["hi_dst"], minlength=NPC)
    csum = np.concatenate(
        [np.zeros((NCORES, 2, 1), np.int64), np.cumsum(cnt, axis=2)], axis=2
    )
    cuts = [0]
    d0 = 0
    while d0 < NPC:
        d1 = min(d0 + NC_CAP, NPC)
        while (csum[:, :, d1] - csum[:, :, d0]).max() > BLK - 1:
            d1 -= 1
        assert d1 > d0
        cuts.append(d1)
        d0 = d1
    NB = len(cuts) - 1

    # per-core static tensors
    IDXM = np.zeros((NCORES, P, NB * (BLK // 16)), np.int16)  # main gather
    ncp = [int(-(-(cuts[b + 1] - cuts[b] + 1) // 32) * 32) for b in range(NB)]
    EOFF = np.cumsum([0] + [n // 16 for n in ncp])
    IDXE = np.zeros((NCORES, P, EOFF[-1]), np.int16)          # boundary gather
    for c in range(NCORES):
        pc = per_core[c]
        for b in range(NB):
            d0, d1 = cuts[b], cuts[b + 1]
            for s, (sk, dk) in enumerate(
                (("lo_src", "lo_dst"), ("hi_src", "hi_dst"))
            ):
                lo_, hi_ = (
                    np.searchsorted(pc[dk], d0),
                    np.searchsorted(pc[dk], d1),
                )
                ssrc = pc[sk][lo_:hi_]
                sdst = pc[dk][lo_:hi_]
                m = hi_ - lo_
                assert m <= BLK - 1
                stream = np.full(BLK, IDX_ZERO, np.int64)
                stream[1 : 1 + m] = ssrc
                w = _wrap16(stream.astype(np.int16), BLK, IDX_ZERO)
                IDXM[c, 64 * s : 64 * (s + 1), b * (BLK // 16) : (b + 1) * (BLK // 16)] = np.tile(w, (4, 1))
                # boundary ends: position of last edge with dst <= d
                ends = np.searchsorted(sdst, np.arange(d0, d1), side="right")
                elist = np.concatenate([[0], ends]).astype(np.int16)  # NC+1
                we = _wrap16(elist, ncp[b], 0)
                IDXE[c, 64 * s : 64 * (s + 1), EOFF[b] : EOFF[b + 1]] = np.tile(we, (4, 1))

    # coefficient tensors per core, feature-major packs
    Dv = np.stack([dinv[c * NPC : (c + 1) * NPC] for c in range(NCORES)])  # [8, 6250]
    return dict(NB=NB, cuts=cuts, ncp=ncp, EOFF=EOFF, IDXM=IDXM, IDXE=IDXE,
                dinv=dinv, Dv=Dv)


def _build_program(meta):
    from concourse import bass, bacc, mybir
    import concourse.tile as tile
    from concourse.masks import make_identity

    f32 = mybir.dt.float32
    bf16 = mybir.dt.bfloat16
    i16 = mybir.dt.int16

    NB = meta["NB"]
    cuts = meta["cuts"]
    ncp = meta["ncp"]
    EOFF = meta["EOFF"]
    ICH = IN_C // P   # 4
    OH = HID_C // P   # 2

    nc = bacc.Bacc(
        "TRN2",
        target_bir_lowering=False,
        debug=False,
        num_devices=NCORES,
        num_swdge_queues=1,
    )

    xT_d = nc.dram_tensor("xT", [IN_C, NPC], f32, kind="ExternalInput")
    W1T_d = nc.dram_tensor("W1T", [IN_C, HID_C], f32, kind="ExternalInput")
    W2T_d = nc.dram_tensor("W2T", [HID_C, OUT_C], f32, kind="ExternalInput")
    b1_d = nc.dram_tensor("b1c", [HID_C, 1], f32, kind="ExternalInput")
    b2_d = nc.dram_tensor("b2c", [OUT_C, 1], f32, kind="ExternalInput")
    IDXM_d = nc.dram_tensor("IDXM", [P, NB * (BLK // 16)], i16, kind="ExternalInput")
    IDXE_d = nc.dram_tensor("IDXE", [P, EOFF[-1]], i16, kind="ExternalInput")
    Dz_d = nc.dram_tensor("Dz", [64, NPC], bf16, kind="ExternalInput")  # dinv rows (own)
    out_d = nc.dram_tensor("out", [NPC, OUT_C], f32, kind="ExternalOutput")

    cc_in = nc.dram_tensor("cc_in", [64, NPC], bf16, kind="Internal")
    hbuf = [
        nc.dram_tensor(f"hbuf{i}", [NCORES * 64, NPC], bf16, kind="Internal",
                       addr_space="Shared")
        for i in range(2)
    ]

    with tile.TileContext(nc) as tc:
        with tc.tile_pool(name="persist", bufs=1) as pp:
            table = pp.tile([P, NELEM], f32)
            idxm = pp.tile([P, NB * (BLK // 16)], i16)
            idxe = pp.tile([P, EOFF[-1]], i16)
            Dt = pp.tile([64, NPC], bf16)     # dinv (own rows, feature-bcast)
            z1t = pp.tile([64, NPC], bf16)    # 0.1*z
            pack_ah = pp.tile([P, NPC], f32)  # [0:64]=aggLo/u/hnew, [64:128]=aggHi
            hs = pp.tile([64, NPC], bf16)     # dinv*h (scaled state)
            ident = pp.tile([OUT_C, OUT_C], f32)

            nc.sync.dma_start(idxm[:], IDXM_d[:, :])
            nc.sync.dma_start(idxe[:], IDXE_d[:, :])
            make_identity(nc, ident[:])
            # zero the table pad region once (slots HALF.. of both halves)
            nc.vector.memset(table[:, HALF:NELEM], 0.0)
            # D (dinv of own rows broadcast over 64 feature partitions)
            nc.sync.dma_start(Dt[:, :], Dz_d[:, :])

            # ---------------- MLP ----------------
            b1_sb = pp.tile([P, OH], f32)
            for i in range(OH):
                nc.sync.dma_start(b1_sb[:, i : i + 1], b1_d[P * i : P * (i + 1), :])
            b2_sb = pp.tile([OUT_C, 1], f32)
            nc.sync.dma_start(b2_sb[:], b2_d[:, :])
            W1T_sb = pp.tile([P, ICH, HID_C], f32)
            for i in range(ICH):
                nc.sync.dma_start(W1T_sb[:, i, :], W1T_d[P * i : P * (i + 1), :])
            W2T_sb = pp.tile([P, OH, OUT_C], f32)
            for i in range(OH):
                nc.sync.dma_start(W2T_sb[:, i, :], W2T_d[P * i : P * (i + 1), :])

            nblk = (NPC + MLP_BLK - 1) // MLP_BLK
            with tc.tile_pool(name="mlp_sb", bufs=2) as mp, tc.tile_pool(
                name="mlp_ps", bufs=2, space="PSUM"
            ) as mps, tc.tile_pool(name="mlp_ps2", bufs=2, space="PSUM") as mps2:
                for b in range(nblk):
                    c0 = b * MLP_BLK
                    W = min(MLP_BLK, NPC - c0)
                    xt = mp.tile([P, ICH, MLP_BLK], f32, tag="xt")
                    for i in range(ICH):
                        nc.sync.dma_start(
                            xt[:, i, :W], xT_d[P * i : P * (i + 1), c0 : c0 + W]
                        )
                    h1 = mp.tile([P, OH, MLP_BLK], f32, tag="h1")
                    for o in range(OH):
                        ps = mps.tile([P, MLP_BLK], f32, tag="psh")
                        for i in range(ICH):
                            nc.tensor.matmul(
                                ps[:, :W],
                                lhsT=W1T_sb[:, i, P * o : P * (o + 1)],
                                rhs=xt[:, i, :W],
                                start=(i == 0),
                                stop=(i == ICH - 1),
                            )
                        nc.scalar.activation(
                            h1[:, o, :W],
                            ps[:, :W],
                            mybir.ActivationFunctionType.Relu,
                            bias=b1_sb[:, o : o + 1],
                        )
                    psz = mps2.tile([OUT_C, MLP_BLK], f32, tag="psz")
                    for o in range(OH):
                        nc.tensor.matmul(
                            psz[:, :W],
                            lhsT=W2T_sb[:, o, :],
                            rhs=h1[:, o, :W],
                            start=(o == 0),
                            stop=(o == OH - 1),
                        )
                    # z block (f32, into agg area as scratch)
                    nc.scalar.activation(
                        pack_ah[0:64, c0 : c0 + W],
                        psz[:, :W],
                        mybir.ActivationFunctionType.Identity,
                        bias=b2_sb[:],
                    )
                    # z1 = 0.1*z (bf16)
                    nc.vector.tensor_scalar_mul(
                        z1t[:, c0 : c0 + W], pack_ah[0:64, c0 : c0 + W], ALPHA
                    )
                    # hs = dinv*z (bf16)
                    nc.vector.tensor_tensor(
                        hs[:, c0 : c0 + W],
                        pack_ah[0:64, c0 : c0 + W],
                        Dt[:, c0 : c0 + W],
                        mybir.AluOpType.mult,
                    )
            nc.sync.dma_start(cc_in[:, :], hs[:, :])
            nc.gpsimd.collective_compute(
                "AllGather",
                mybir.AluOpType.bypass,
                replica_groups=[list(range(NCORES))],
                ins=[cc_in[:, :].opt()],
                outs=[hbuf[0][:, :].opt()],
            )

            # ---------------- propagation ----------------
            with tc.tile_pool(name="gp", bufs=2) as gp, tc.tile_pool(
                name="sp", bufs=2
            ) as sp, tc.tile_pool(name="ep", bufs=2) as ep:
                for k in range(K):
                    hb = hbuf[k % 2]
                    # reload table from hbuf (bf16 -> f32 cast DMA)
                    for c in range(NCORES):
                        half = 0 if c < 4 else 64
                        co = (c % 4) * NPC
                        nc.gpsimd.dma_start(
                            table[half : half + 64, co : co + NPC],
                            hb[64 * c : 64 * (c + 1), :],
                        )
                    for b in range(NB):
                        d0, d1 = cuts[b], cuts[b + 1]
                        ncb = d1 - d0
                        G = gp.tile([P, BLK], f32, tag="G")
                        nc.gpsimd.ap_gather(
                            G[:],
                            table[:],
                            idxm[:, b * (BLK // 16) : (b + 1) * (BLK // 16)],
                            channels=P,
                            num_elems=NELEM,
                            d=1,
                            num_idxs=BLK,
                        )
                        Pt = sp.tile([P, BLK], f32, tag="P")
                        nc.vector.tensor_tensor_scan(
                            Pt[:],
                            G[:],
                            G[:],
                            0.0,
                            mybir.AluOpType.add,
                            mybir.AluOpType.bypass,
                        )
                        Et = ep.tile([P, NC_CAP + 16], f32, tag="E")
                        ncb16 = ncp[b]
                        nc.gpsimd.ap_gather(
                            Et[:, :ncb16],
                            Pt[:],
                            idxe[:, EOFF[b] : EOFF[b + 1]],
                            channels=P,
                            num_elems=BLK,
                            d=1,
                            num_idxs=ncb16,
                        )
                        nc.vector.tensor_tensor(
                            pack_ah[:, d0:d1],
                            Et[:, 1 : ncb + 1],
                            Et[:, 0:ncb],
                            mybir.AluOpType.subtract,
                        )
                    # aggLo += aggHi (partition shift via DMA copy + DVE add)
                    for cc in range(0, NPC, 1024):
                        w = min(1024, NPC - cc)
                        scr = sp.tile([P, BLK], f32, tag="scr")
                        nc.sync.dma_start(
                            scr[0:64, :w], pack_ah[64:128, cc : cc + w]
                        )
                        nc.vector.tensor_tensor(
                            pack_ah[0:64, cc : cc + w],
                            pack_ah[0:64, cc : cc + w],
                            scr[0:64, :w],
                            mybir.AluOpType.add,
                        )
                    # mix: hnew = 0.9*D*(agg + hs) + z1 ; hs' = D*hnew
                    nc.vector.tensor_tensor(
                        pack_ah[0:64, :],
                        pack_ah[0:64, :],
                        hs[:, :],
                        mybir.AluOpType.add,
                    )
                    nc.vector.tensor_tensor(
                        pack_ah[0:64, :],
                        pack_ah[0:64, :],
                        Dt[:, :],
                        mybir.AluOpType.mult,
                    )
                    nc.vector.scalar_tensor_tensor(
                        pack_ah[0:64, :],
                        pack_ah[0:64, :],
                        1.0 - ALPHA,
                        z1t[:, :],
                        op0=mybir.AluOpType.mult,
                        op1=mybir.AluOpType.add,
                    )
                    last = k == K - 1
                    if not last:
                        nc.vector.tensor_tensor(
                            hs[:, :],
                            pack_ah[0:64, :],
                            Dt[:, :],
                            mybir.AluOpType.mult,
                        )
                        nc.sync.dma_start(cc_in[:, :], hs[:, :])
                        nc.gpsimd.collective_compute(
                            "AllGather",
                            mybir.AluOpType.bypass,
                            replica_groups=[list(range(NCORES))],
                            ins=[cc_in[:, :].opt()],
                            outs=[hbuf[(k + 1) % 2][:, :].opt()],
                        )
                with tc.tile_pool(name="tr", bufs=2, space="PSUM") as tps, tc.tile_pool(
                    name="trs", bufs=2
                ) as trs:
                    for j in range((NPC + P - 1) // P):
                        r = min(P, NPC - j * P)
                        ptr = tps.tile([P, OUT_C], f32, tag="ptr")
                        nc.tensor.transpose(
                            ptr[:r, :], pack_ah[0:64, j * P : j * P + r], ident[:]
                        )
                        hr = trs.tile([P, OUT_C], f32, tag="hr")
                        nc.scalar.copy(hr[:r, :], ptr[:r, :])
                        nc.sync.dma_start(out_d[j * P : j * P + r, :], hr[:r, :])

    nc.compile()
    return nc


# ---------------------------------------------------------------- runner

_CACHE = {}


def _get_program(edge_index):
    key = ("prog2", int(np.asarray(edge_index).sum() & 0xFFFFFFFF))
    if key not in _CACHE:
        meta = _prep(edge_index)
        nc = _build_program(meta)
        _CACHE[key] = (nc, meta)
    return _CACHE[key]


def kernel(x, edge_index, W1, b1, W2, b2):
    x = np.ascontiguousarray(np.asarray(x, dtype=np.float32))
    edge_index = np.asarray(edge_index)
    W1 = np.asarray(W1, dtype=np.float32)
    b1 = np.asarray(b1, dtype=np.float32)
    W2 = np.asarray(W2, dtype=np.float32)
    b2 = np.asarray(b2, dtype=np.float32)

    nc, meta = _get_program(edge_index)

    W1T = np.ascontiguousarray(W1.T)
    W2T = np.ascontiguousarray(W2.T)
    b1c = np.ascontiguousarray(b1.reshape(-1, 1))
    b2c = np.ascontiguousarray(b2.reshape(-1, 1))

    in_maps = []
    import ml_dtypes

    for c in range(NCORES):
        xT_c = np.ascontiguousarray(x[c * NPC : (c + 1) * NPC].T)
        Dz = np.ascontiguousarray(
            np.broadcast_to(meta["Dv"][c][None, :], (64, NPC)).astype(ml_dtypes.bfloat16)
        )
        in_maps.append(
            dict(
                xT=xT_c, W1T=W1T, W2T=W2T, b1c=b1c, b2c=b2c,
                IDXM=np.ascontiguousarray(meta["IDXM"][c]),
                IDXE=np.ascontiguousarray(meta["IDXE"][c]),
                Dz=Dz,
            )
        )

    from concourse import bass_utils

    res = bass_utils.run_bass_kernel_spmd(
        nc, in_maps, core_ids=list(range(NCORES)), trace=bool(os.environ.get("APPNP_TRACE"))
    )
    out = np.concatenate([res.results[c]["out"] for c in range(NCORES)], axis=0)
    kernel.last_exec_time_ns = res.exec_time_ns
    kernel.last_results = res
    return out


# revision 13
# speedup vs baseline: 1.8645x; 1.8645x over previous
"""APPNP node classifier on 8 TRN2 NeuronCores.

Strategy (graph/data parallel, per sharding hint):
  - Nodes sharded across 8 cores (6250 nodes/core); MLP weights replicated.
  - MLP (x @ W1.T -> relu -> @ W2.T) computed feature-major (zT = [64, nodes]).
  - 10 APPNP propagation steps. Each step:
      * AllGather the per-core h rows -> full h [50000, 64] in HBM (per core copy)
      * dma_gather (SWDGE) fetches h[src] rows (256B each) for this core's edges,
        in a host-precomputed chunk order (128 edges/chunk)
      * per chunk, one PE matmul with a host-built scatter matrix S [128, 16]
        (edge weight at the edge's destination column) accumulates the
        segment sum into PSUM agg tiles [64, 128] (feature-major)
      * alpha mix hnew = 0.9*agg + 0.1*z fused on DVE, PE-transpose back to
        row-major, DMA rows out.
  - Self-loops handled as extra edges with weight deg^-1/2 * deg^-1/2.
  - int16 gather indices only reach 32767, so edges are split into two gather
    calls per group: src < 32768 (base row 0) and src >= 32768 (base row 32768).

The chunk schedule (window starts, chunk counts) is baked into the single SPMD
program, computed as a max over all 8 cores; each core's S / index data pads
its unused chunk slots with index 0 and zero weights.
"""

import os
import sys
import types

for _p in ("/opt/trn_rl_repo", "/root/.axon_site/_ro/trn_rl_repo", "/root/.axon_site"):
    if os.path.isdir(_p) and _p not in sys.path:
        sys.path.insert(0, _p)

import numpy as np

# ---------------------------------------------------------------- config

FULL_CFG = dict(
    N=50000,
    IN_C=512,
    HID_C=256,
    OUT_C=64,
    K=10,
    ALPHA=0.1,
    HALF=32768,
    WMAX=16,
    GT=4,  # dst-tiles per gather group
    MLP_BLK=512,
)

NCORES = 8
P = 128


# ---------------------------------------------------------------- host preprocessing


def _schedule_and_tensors(edge_index, cfg):
    """Build the baked chunk schedule and per-core S / index tensors."""
    N = cfg["N"]
    NPC = N // NCORES
    HALF = cfg["HALF"]
    WMAX = cfg["WMAX"]
    TILES = (NPC + P - 1) // P

    src = np.asarray(edge_index[0], dtype=np.int64)
    dst = np.asarray(edge_index[1], dtype=np.int64)

    deg = np.bincount(dst, minlength=N).astype(np.float64) + 1.0
    dinv = 1.0 / np.sqrt(deg)
    w_e = (dinv[src] * dinv[dst]).astype(np.float32)

    ar = np.arange(N, dtype=np.int64)
    all_src = np.concatenate([src, ar])
    all_dst = np.concatenate([dst, ar])
    all_w = np.concatenate([w_e, (dinv * dinv).astype(np.float32)])

    core = all_dst // NPC
    ld = all_dst % NPC
    tile_id = ld // P
    col = ld % P
    half = (all_src >= HALF).astype(np.int64)

    key = ((core * TILES + tile_id) * 2 + half) * P + col
    counts = np.bincount(key, minlength=NCORES * TILES * 2 * P).reshape(
        NCORES, TILES, 2, P
    )

    # greedy windows per (tile, half): whole-dst columns, cap 128 edges for the
    # worst core, window width <= WMAX
    schedule = {}
    for t in range(TILES):
        for h in (0, 1):
            c = counts[:, t, h, :]  # [8, P]
            csum = np.cumsum(c, axis=1)
            chunks_th = []
            s = 0
            while s < P:
                if c[:, s].max() == 0:
                    s += 1
                    continue
                base = csum[:, s - 1] if s > 0 else np.zeros(NCORES, np.int64)
                e = s
                while e < min(s + WMAX, P) and (csum[:, e] - base).max() <= P:
                    e += 1
                assert e > s, f"dst column with >128 edges at tile {t}"
                chunks_th.append((s, e))
                s = e
            schedule[(t, h)] = chunks_th

    GT = cfg["GT"]
    NGROUPS = (TILES + GT - 1) // GT
    chunk_order = []  # (t, h, s, e)
    groups = []  # per group: dict(start, nA, nB, tiles)
    for g in range(NGROUPS):
        tiles_g = list(range(g * GT, min((g + 1) * GT, TILES)))
        a = [(t, 0, s, e) for t in tiles_g for (s, e) in schedule[(t, 0)]]
        b = [(t, 1, s, e) for t in tiles_g for (s, e) in schedule[(t, 1)]]
        groups.append(dict(start=len(chunk_order), nA=len(a), nB=len(b), tiles=tiles_g))
        chunk_order += a + b
    NCHUNK = len(chunk_order)

    # map (tile, half, col) -> global chunk id + window start
    chunk_of = np.full((TILES, 2, P), -1, np.int64)
    s_of = np.zeros(NCHUNK, np.int64)
    for cg, (t, h, s, e) in enumerate(chunk_order):
        chunk_of[t, h, s:e] = cg
        s_of[cg] = s

    cg_e = chunk_of[tile_id, half, col]
    assert (cg_e >= 0).all()

    # slot within (core, chunk)
    okey = core * NCHUNK + cg_e
    order = np.argsort(okey, kind="stable")
    sk = okey[order]
    is_start = np.ones(len(sk), bool)
    is_start[1:] = sk[1:] != sk[:-1]
    grp_start = np.maximum.accumulate(np.where(is_start, np.arange(len(sk)), 0))
    slot = np.arange(len(sk)) - grp_start
    assert slot.max() < P

    e_core = core[order]
    e_cg = cg_e[order]
    e_src = all_src[order]
    e_w = all_w[order]
    e_col = col[order]
    e_half = half[order]

    IDX = np.zeros((NCORES, NCHUNK, P), np.int64)
    S = np.zeros((NCORES, NCHUNK, P, WMAX), np.float32)
    IDX[e_core, e_cg, slot] = e_src - HALF * e_half
    S[e_core, e_cg, slot, e_col - s_of[e_cg]] = e_w
    assert IDX.max() < 32768 and IDX.min() >= 0

    # wrap indices: slot i of chunk -> partition i%16, col chunk*8 + i//16
    IDXw = (
        IDX.astype(np.int16)
        .reshape(NCORES, NCHUNK, P // 16, 16)
        .transpose(0, 3, 1, 2)
        .reshape(NCORES, 16, NCHUNK * (P // 16))
    )
    IDXw = np.tile(IDXw, (1, 8, 1))  # replicate across the 8 gpsimd cores

    Sw = S.transpose(0, 2, 1, 3).reshape(NCORES, P, NCHUNK * WMAX)

    meta = dict(
        NPC=NPC,
        TILES=TILES,
        NCHUNK=NCHUNK,
        NGROUPS=NGROUPS,
        groups=groups,
        chunk_order=chunk_order,
        WMAX=WMAX,
    )
    return meta, Sw, IDXw


# ---------------------------------------------------------------- device program


def _build_program(cfg, meta):
    from concourse import bass, bacc, mybir
    import concourse.tile as tile
    from concourse.masks import make_identity

    f32 = mybir.dt.float32
    i16 = mybir.dt.int16

    N = cfg["N"]
    IN_C = cfg["IN_C"]
    HID_C = cfg["HID_C"]
    OUT_C = cfg["OUT_C"]
    K = cfg["K"]
    ALPHA = cfg["ALPHA"]
    HALF = cfg["HALF"]
    WMAX = cfg["WMAX"]
    NPC = meta["NPC"]
    TILES = meta["TILES"]
    NCHUNK = meta["NCHUNK"]
    groups = meta["groups"]
    chunk_order = meta["chunk_order"]
    MLP_BLK = cfg["MLP_BLK"]
    ICH = IN_C // P  # input-feature chunks of 128
    OH = HID_C // P  # hidden halves of 128

    nc = bacc.Bacc(
        "TRN2",
        target_bir_lowering=False,
        debug=False,
        num_devices=NCORES,
        num_swdge_queues=4,
    )

    xT_d = nc.dram_tensor("xT", [IN_C, NPC], f32, kind="ExternalInput")
    W1T_d = nc.dram_tensor("W1T", [IN_C, HID_C], f32, kind="ExternalInput")
    W2T_d = nc.dram_tensor("W2T", [HID_C, OUT_C], f32, kind="ExternalInput")
    b1_d = nc.dram_tensor("b1c", [HID_C, 1], f32, kind="ExternalInput")
    b2_d = nc.dram_tensor("b2c", [OUT_C, 1], f32, kind="ExternalInput")
    S_d = nc.dram_tensor("Sw", [P, NCHUNK * WMAX], f32, kind="ExternalInput")
    IDX_d = nc.dram_tensor("IDXw", [P, NCHUNK * 8], i16, kind="ExternalInput")
    out_d = nc.dram_tensor("out", [NPC, OUT_C], f32, kind="ExternalOutput")

    hbuf = [
        nc.dram_tensor(f"hfull{i}", [N, OUT_C], f32, kind="Internal", addr_space="Shared")
        for i in range(2)
    ]
    rows_b = nc.dram_tensor("rows_b", [NPC, OUT_C], f32, kind="Internal")

    with tile.TileContext(nc) as tc:
        with tc.tile_pool(name="persist", bufs=1) as pp:
            S_sb = pp.tile([P, NCHUNK, WMAX], f32)
            nc.sync.dma_start(S_sb[:].rearrange("p a b -> p (a b)"), S_d[:, :])
            idx_sb = pp.tile([P, NCHUNK * 8], i16)
            nc.sync.dma_start(idx_sb[:], IDX_d[:, :])
            W1T_sb = pp.tile([P, ICH, HID_C], f32)
            for i in range(ICH):
                nc.sync.dma_start(W1T_sb[:, i, :], W1T_d[P * i : P * (i + 1), :])
            W2T_sb = pp.tile([P, OH, OUT_C], f32)
            for i in range(OH):
                nc.sync.dma_start(W2T_sb[:, i, :], W2T_d[P * i : P * (i + 1), :])
            b1_sb = pp.tile([P, OH], f32)
            for i in range(OH):
                nc.sync.dma_start(b1_sb[:, i : i + 1], b1_d[P * i : P * (i + 1), :])
            b2_sb = pp.tile([OUT_C, 1], f32)
            nc.sync.dma_start(b2_sb[:], b2_d[:, :])
            b2s_sb = pp.tile([OUT_C, 1], f32)
            nc.scalar.mul(b2s_sb[:], b2_sb[:], ALPHA)
            ident = pp.tile([OUT_C, OUT_C], f32)
            make_identity(nc, ident[:])
            zTs = pp.tile([OUT_C, TILES * P], f32)  # ALPHA * z, feature-major
            nc.vector.memset(zTs[:], 0.0)

            # ---------------- MLP ----------------
            nblk = (NPC + MLP_BLK - 1) // MLP_BLK
            with tc.tile_pool(name="mlp_sb", bufs=2) as mp, tc.tile_pool(
                name="mlp_ps", bufs=2, space="PSUM"
            ) as mps, tc.tile_pool(name="mlp_ps2", bufs=2, space="PSUM") as mps2, tc.tile_pool(
                name="mlp_tr", bufs=2, space="PSUM"
            ) as mtr:
                for b in range(nblk):
                    c0 = b * MLP_BLK
                    W = min(MLP_BLK, NPC - c0)
                    xt = mp.tile([P, ICH, MLP_BLK], f32, tag="xt")
                    for i in range(ICH):
                        nc.sync.dma_start(
                            xt[:, i, :W], xT_d[P * i : P * (i + 1), c0 : c0 + W]
                        )
                    h1 = mp.tile([P, OH, MLP_BLK], f32, tag="h1")
                    for o in range(OH):
                        ps = mps.tile([P, MLP_BLK], f32, tag="psh")
                        for i in range(ICH):
                            nc.tensor.matmul(
                                ps[:, :W],
                                lhsT=W1T_sb[:, i, P * o : P * (o + 1)],
                                rhs=xt[:, i, :W],
                                start=(i == 0),
                                stop=(i == ICH - 1),
                            )
                        nc.scalar.activation(
                            h1[:, o, :W],
                            ps[:, :W],
                            mybir.ActivationFunctionType.Relu,
                            bias=b1_sb[:, o : o + 1],
                        )
                    psz = mps2.tile([OUT_C, MLP_BLK], f32, tag="psz")
                    for o in range(OH):
                        nc.tensor.matmul(
                            psz[:, :W],
                            lhsT=W2T_sb[:, o, :],
                            rhs=h1[:, o, :W],
                            start=(o == 0),
                            stop=(o == OH - 1),
                        )
                    # scaled copy for the mix, and plain rows for h0
                    nc.scalar.activation(
                        zTs[:, c0 : c0 + W],
                        psz[:, :W],
                        mybir.ActivationFunctionType.Identity,
                        bias=b2s_sb[:],
                        scale=ALPHA,
                    )
                    zp = mp.tile([OUT_C, MLP_BLK], f32, tag="zp")
                    nc.scalar.activation(
                        zp[:, :W],
                        psz[:, :W],
                        mybir.ActivationFunctionType.Identity,
                        bias=b2_sb[:],
                    )
                    rows_t0 = out_d if cfg.get("MLP_ONLY") else rows_b
                    for j in range((W + P - 1) // P):
                        r = min(P, W - j * P)
                        ptr = mtr.tile([P, OUT_C], f32, tag="ptr")
                        nc.tensor.transpose(
                            ptr[:r, :], zp[:, j * P : j * P + r], ident[:]
                        )
                        zr = mp.tile([P, OUT_C], f32, tag="zr")
                        nc.scalar.copy(zr[:r, :], ptr[:r, :])
                        nc.sync.dma_start(
                            rows_t0[c0 + j * P : c0 + j * P + r, :], zr[:r, :]
                        )
            if cfg.get("MLP_ONLY"):
                pass
            else:
                _prop(
                    nc, tc, cfg, meta, hbuf, rows_b, out_d, idx_sb, S_sb, zTs, ident
                )

    nc.compile()
    return nc


def _prop(nc, tc, cfg, meta, hbuf, rows_b, out_d, idx_sb, S_sb, zTs, ident):
    from concourse import mybir
    import concourse.tile as tile

    f32 = mybir.dt.float32
    N = cfg["N"]
    OUT_C = cfg["OUT_C"]
    K = cfg["K"]
    ALPHA = cfg["ALPHA"]
    HALF = cfg["HALF"]
    WMAX = cfg["WMAX"]
    NPC = meta["NPC"]
    groups = meta["groups"]
    chunk_order = meta["chunk_order"]
    NCORES = 8

    if True:
            nc.gpsimd.collective_compute(
                "AllGather",
                mybir.AluOpType.bypass,
                replica_groups=[list(range(NCORES))],
                ins=[rows_b[:, :].opt()],
                outs=[hbuf[0][:, :].opt()],
            )

            # ---------------- propagation ----------------
            with tc.tile_pool(name="prop_sb", bufs=8) as gp, tc.tile_pool(
                name="prop_sb2", bufs=3
            ) as hp, tc.tile_pool(name="prop_ps", bufs=4, space="PSUM") as aps, tc.tile_pool(
                name="prop_tr", bufs=2, space="PSUM"
            ) as tps:
                for step in range(K):
                    h_src = hbuf[step % 2]
                    last = step == K - 1
                    rows_target = out_d if last else rows_b
                    qrr = 0
                    for g in groups:
                        nA, nB = g["nA"], g["nB"]
                        c0 = g["start"]
                        ng = nA + nB
                        # SWDGE ring holds 1024 descriptors -> max 8 chunks
                        # (1024 rows) per dma_gather call; round-robin the 4
                        # queues so the Q7 DSP pairs generate in parallel.
                        # Slab call boundaries (single-half each):
                        slab_calls = []
                        for lo, hi in ((c0, c0 + nA), (c0 + nA, c0 + ng)):
                            cc = lo
                            while cc < hi:
                                n = min(8, hi - cc)
                                slab_calls.append((cc, n, lo == c0))
                                cc += n
                        # psum agg per tile of the group + remaining-chunk count
                        aggs, remaining = {}, {}
                        for t in g["tiles"]:
                            aggs[t] = aps.tile(
                                [OUT_C, P + WMAX], f32, tag="agg", name=f"agg{t}"
                            )
                            nc.vector.memset(aggs[t][:], 0.0)
                            remaining[t] = 0
                        for lc in range(ng):
                            remaining[chunk_order[c0 + lc][0]] += 1
                        # issue each slab's gather, then immediately its
                        # matmuls (slab-order consumption so G slots recycle
                        # in gather order and many DMAs stay in flight)
                        for cc, n, is_a in slab_calls:
                            src_ap = h_src[0:HALF, :] if is_a else h_src[HALF:N, :]
                            G = gp.tile([P, 8, OUT_C], f32, tag="G")
                            nc.gpsimd.dma_gather(
                                G[:, :n, :],
                                src_ap,
                                idx_sb[:, 8 * cc : 8 * (cc + n)],
                                num_idxs=P * n,
                                num_idxs_reg=P * n,
                                elem_size=OUT_C,
                                queue_num=qrr % 4,
                            )
                            qrr += 1
                            for j in range(n):
                                t, h, s, e = chunk_order[cc + j]
                                nc.tensor.matmul(
                                    aggs[t][:, s : s + WMAX],
                                    lhsT=G[:, j, :],
                                    rhs=S_sb[:, cc + j, :],
                                    start=False,
                                    stop=True,
                                    skip_group_check=True,
                                )
                                remaining[t] -= 1
                                if remaining[t] == 0:
                                    ht = hp.tile([OUT_C, P], f32, tag="ht")
                                    nc.vector.scalar_tensor_tensor(
                                        ht[:],
                                        aggs[t][:, :P],
                                        1.0 - ALPHA,
                                        zTs[:, t * P : (t + 1) * P],
                                        op0=mybir.AluOpType.mult,
                                        op1=mybir.AluOpType.add,
                                    )
                                    r = min(P, NPC - t * P)
                                    ptr = tps.tile([P, OUT_C], f32, tag="ptr2")
                                    nc.tensor.transpose(
                                        ptr[:r, :], ht[:, :r], ident[:]
                                    )
                                    hr = hp.tile([P, OUT_C], f32, tag="hr")
                                    nc.scalar.copy(hr[:r, :], ptr[:r, :])
                                    nc.sync.dma_start(
                                        rows_target[t * P : t * P + r, :],
                                        hr[:r, :],
                                    )
                    if not last:
                        nc.gpsimd.collective_compute(
                            "AllGather",
                            mybir.AluOpType.bypass,
                            replica_groups=[list(range(NCORES))],
                            ins=[rows_b[:, :].opt()],
                            outs=[hbuf[(step + 1) % 2][:, :].opt()],
                        )


# ---------------------------------------------------------------- runner

_CACHE = {}


def _get_program(edge_index, cfg):
    key = ("prog", cfg["N"], int(np.asarray(edge_index).sum() & 0xFFFFFFFF))
    if key not in _CACHE:
        meta, Sw, IDXw = _schedule_and_tensors(edge_index, cfg)
        nc = _build_program(cfg, meta)
        _CACHE[key] = (nc, meta, Sw, IDXw)
    return _CACHE[key]


def kernel(x, edge_index, W1, b1, W2, b2, _cfg=None):
    cfg = dict(FULL_CFG if _cfg is None else _cfg)
    x = np.ascontiguousarray(np.asarray(x, dtype=np.float32))
    edge_index = np.asarray(edge_index)
    W1 = np.asarray(W1, dtype=np.float32)
    b1 = np.asarray(b1, dtype=np.float32)
    W2 = np.asarray(W2, dtype=np.float32)
    b2 = np.asarray(b2, dtype=np.float32)

    nc, meta, Sw, IDXw = _get_program(edge_index, cfg)
    NPC = meta["NPC"]

    W1T = np.ascontiguousarray(W1.T)
    W2T = np.ascontiguousarray(W2.T)
    b1c = np.ascontiguousarray(b1.reshape(-1, 1))
    b2c = np.ascontiguousarray(b2.reshape(-1, 1))

    in_maps = []
    for c in range(NCORES):
        xT_c = np.ascontiguousarray(x[c * NPC : (c + 1) * NPC].T)
        in_maps.append(
            dict(
                xT=xT_c,
                W1T=W1T,
                W2T=W2T,
                b1c=b1c,
                b2c=b2c,
                Sw=np.ascontiguousarray(Sw[c]),
                IDXw=np.ascontiguousarray(IDXw[c]),
            )
        )

    from concourse import bass_utils

    res = bass_utils.run_bass_kernel_spmd(
        nc, in_maps, core_ids=list(range(NCORES)), trace=bool(os.environ.get("APPNP_TRACE"))
    )
    out = np.concatenate([res.results[c]["out"] for c in range(NCORES)], axis=0)
    kernel.last_exec_time_ns = res.exec_time_ns
    kernel.last_results = res
    return out

